# revision 26
# baseline (speedup 1.0000x reference)
"""Trainium2 Bass kernel for per-sample generated low-rank linear:

    h   = inp @ U                      # [B, 128] -> [B, 32]
    h2  = einsum('bi,bio->bo', h, gen_weight.reshape(B, 32, 32))
    out = h2 @ V + bias                # [B, 32] -> [B, 128]

Strategy: pure data parallel over 8 NeuronCores (B rows split evenly).
Per 128-row tile (batch b in partitions):
  PE:   h_rep = inpT.T @ U_rep (float32r, U columns each repeated 32x)
        so PSUM holds h_rep[b, 32i+o] = h[b, i] -- the per-sample GEMV
        becomes a flat elementwise multiply with gen_weight.
  DVE:  tmp = gw * h_rep (bf16 out) + tree-add levels 2-4 (i-major
        halving keeps the 32 o-lanes aligned through every level).
  Pool: tree levels 1 and 5 (SBUF-only; Pool cannot read PSUM). The
        final level writes the tile's h2 into a shared quad tile.
  PE:   4 tiles' h2 transposed in one shot, then one matmul against a
        block-diagonal V (plus a K=1 ones x bias_rep matmul) puts all
        4 tiles' outputs + bias in one [128, 512] PSUM bank.
  ACT:  quad-transpose and out4 PSUM->SBUF copies; issues inp/out DMAs
        (HWDGE); gw chunk DMAs alternate between the SP and ACT rings.

Host-side prep (part of kernel()): shard rows, transpose the inp shard
to [128, BL] (feature-major: contraction dim = partition dim on-chip,
4KB contiguous DMA runs), regroup gen_weight to [P, NTILES, 1024]
(32KB contiguous runs per partition), build U_rep / block-diagonal V /
replicated bias, and un-permute the [P, NTILES, F] device output.
"""

import sys

if "/opt/trn_rl_repo" not in sys.path:
    sys.path.insert(0, "/opt/trn_rl_repo")

import numpy as np

B = 131072
IN_FEAT = 128
OUT_FEAT = 128
RANK = 32
N_CORES = 8
BL = B // N_CORES          # rows per core
P = 128                    # partitions / rows per tile
NTILES = BL // P           # 128 tiles per core
CH = 8                     # tiles per DMA chunk
NCH = NTILES // CH
QD = 4                     # tiles per output quad

_cached = {}


def _build_nc():
    from concourse import bacc, masks, mybir
    from concourse.tile import TileContext

    f32 = mybir.dt.float32
    f32r = mybir.dt.float32r
    bf16 = mybir.dt.bfloat16
    Alu = mybir.AluOpType
    RR = RANK * RANK

    nc = bacc.Bacc(None)
    inp_e = nc.declare_dram_parameter("inp", [IN_FEAT, BL], f32r, isOutput=False)
    gw_e = nc.declare_dram_parameter(
        "gen_weight", [P, NTILES, RR], f32, isOutput=False
    )
    urep_e = nc.declare_dram_parameter("u_rep", [IN_FEAT, RR], f32r, isOutput=False)
    vblk_e = nc.declare_dram_parameter(
        "v_blk", [QD * RANK, QD * OUT_FEAT], f32, isOutput=False
    )
    biasr_e = nc.declare_dram_parameter(
        "bias_rep", [1, QD * OUT_FEAT], f32, isOutput=False
    )
    out_e = nc.declare_dram_parameter(
        "out", [P, NTILES, OUT_FEAT], f32, isOutput=True
    )

    with TileContext(nc) as tc:
        with (
            tc.tile_pool(name="const", bufs=1) as cpool,
            tc.tile_pool(name="io", bufs=2) as io,
            tc.tile_pool(name="gwp", bufs=3) as gwp,
            tc.tile_pool(name="work", bufs=3) as work,
            tc.tile_pool(name="quad", bufs=2) as quad,
            tc.tile_pool(name="pH", bufs=2, space="PSUM") as pH,
            tc.tile_pool(name="pS", bufs=2, space="PSUM") as pS,
            tc.tile_pool(name="pO", bufs=2, space="PSUM") as pO,
        ):
            ident = cpool.tile([P, P], bf16)
            masks.make_identity(nc, ident[:])
            urep_sb = cpool.tile([IN_FEAT, RR], f32r)
            nc.sync.dma_start(urep_sb[:], urep_e[:])
            vblk_sb = cpool.tile([QD * RANK, QD * OUT_FEAT], bf16)
            nc.gpsimd.dma_start(vblk_sb[:], vblk_e[:])  # SWDGE casts to bf16
            biasr_sb = cpool.tile([1, QD * OUT_FEAT], bf16)
            nc.gpsimd.dma_start(biasr_sb[:], biasr_e[:])
            ones_sb = cpool.tile([1, P], bf16)
            nc.vector.memset(ones_sb[:], 1.0)

            for c in range(NCH):
                inpT = io.tile([P, CH, P], f32r, tag="inpT")
                nc.scalar.dma_start(inpT[:], inp_e[:, c * CH * P : (c + 1) * CH * P])
                gw_c = gwp.tile([P, CH, RR], f32, tag="gw")
                eng = nc.sync if (c % 2 == 0) else nc.scalar
                eng.dma_start(gw_c[:], gw_e[:, c * CH : (c + 1) * CH, :])
                out_c = io.tile([P, CH, OUT_FEAT], f32, tag="out")

                for q in range(CH // QD):
                    h2q = quad.tile([P, QD * RANK], bf16, tag="h2q")
                    for tq in range(QD):
                        t = q * QD + tq
                        # h_rep[b, 32i+o] = h[b, i] via U_rep (f32r)
                        hrep = pH.tile([P, RR], f32, tag="hrep")
                        nc.tensor.matmul(
                            hrep[:, 0:512], inpT[:, t, :], urep_sb[:, 0:512]
                        )
                        nc.tensor.matmul(
                            hrep[:, 512:1024], inpT[:, t, :], urep_sb[:, 512:1024]
                        )

                        # ACT stages the high half of h_rep into SBUF so
                        # Pool (no PSUM access) can multiply it
                        hh_sb = work.tile([P, 512], f32, tag="hh")
                        nc.scalar.copy(hh_sb[:], hrep[:, 512:1024])

                        # tmp = gw * h_rep: DVE low half (PSUM direct),
                        # Pool high half; then i-major halving tree
                        tmp = work.tile([P, RR], bf16, tag="tmp")
                        nc.vector.tensor_tensor(
                            tmp[:, 0:512], gw_c[:, t, 0:512], hrep[:, 0:512],
                            Alu.mult
                        )
                        nc.gpsimd.tensor_tensor(
                            tmp[:, 512:1024], gw_c[:, t, 512:1024], hh_sb[:],
                            Alu.mult
                        )
                        nc.vector.tensor_tensor(
                            tmp[:, 0:512], tmp[:, 0:512], tmp[:, 512:1024], Alu.add
                        )
                        nc.vector.tensor_tensor(
                            tmp[:, 0:256], tmp[:, 0:256], tmp[:, 256:512], Alu.add
                        )
                        nc.vector.tensor_tensor(
                            tmp[:, 0:128], tmp[:, 0:128], tmp[:, 128:256], Alu.add
                        )
                        nc.vector.tensor_tensor(
                            tmp[:, 0:64], tmp[:, 0:64], tmp[:, 64:128], Alu.add
                        )
                        nc.gpsimd.tensor_tensor(
                            h2q[:, tq * RANK : (tq + 1) * RANK],
                            tmp[:, 0:32],
                            tmp[:, 32:64],
                            Alu.add,
                        )

                    # quad: transpose 4 tiles' h2 at once, one block-diag
                    # V matmul + ones x bias_rep -> 4 tiles' outputs
                    psQ = pS.tile([QD * RANK, P], bf16, tag="qT")
                    nc.tensor.transpose(psQ[:], h2q[:], ident[:])
                    qT = quad.tile([QD * RANK, P], bf16, tag="qT_sb")
                    nc.scalar.copy(qT[:], psQ[:])

                    out4 = pO.tile([P, QD * OUT_FEAT], f32, tag="out4")
                    nc.tensor.matmul(out4[:], qT[:], vblk_sb[:], start=True, stop=False)
                    nc.tensor.matmul(
                        out4[:], ones_sb[:], biasr_sb[:], start=False, stop=True
                    )
                    nc.scalar.copy(
                        out_c[:, q * QD : (q + 1) * QD, :].rearrange(
                            "p t o -> p (t o)"
                        ),
                        out4[:],
                    )

                nc.scalar.dma_start(out_e[:, c * CH : (c + 1) * CH, :], out_c[:])

    nc.compile()
    return nc


def _get_nc():
    if "nc" not in _cached:
        _cached["nc"] = _build_nc()
    return _cached["nc"]


def run(inputs, trace=False):
    """Returns (full_output [B, OUT_FEAT] fp32, BassKernelResults)."""
    from concourse.bass_utils import run_bass_kernel_spmd

    inp = np.ascontiguousarray(inputs["inp"], dtype=np.float32)
    gw = np.ascontiguousarray(inputs["gen_weight"], dtype=np.float32)
    u = np.ascontiguousarray(inputs["U"], dtype=np.float32)
    v = np.ascontiguousarray(inputs["V"], dtype=np.float32)
    bias = np.ascontiguousarray(inputs["bias"], dtype=np.float32)

    v_blk = np.zeros((QD * RANK, QD * OUT_FEAT), dtype=np.float32)
    for qd in range(QD):
        v_blk[qd * RANK : (qd + 1) * RANK, qd * OUT_FEAT : (qd + 1) * OUT_FEAT] = v
    bias_rep = np.tile(bias.reshape(1, OUT_FEAT), (1, QD))

    in_maps = []
    for i in range(N_CORES):
        sl = slice(i * BL, (i + 1) * BL)
        # regroup: gw2[p, n, :] = gw[n*128+p, :]  (i-major kept)
        g = gw[sl].reshape(NTILES, P, RANK * RANK)
        g2 = np.ascontiguousarray(g.transpose(1, 0, 2))
        in_maps.append(
            {
                "inp": np.ascontiguousarray(inp[sl].T),
                "gen_weight": g2,
                "u_rep": np.repeat(u, RANK, axis=1),
                "v_blk": v_blk,
                "bias_rep": bias_rep,
            }
        )

    nc = _get_nc()
    res = run_bass_kernel_spmd(nc, in_maps, core_ids=list(range(N_CORES)), trace=trace)
    # device layout [P, NTILES, F]: sample s = n*128 + p
    shards = [
        r["out"].transpose(1, 0, 2).reshape(BL, OUT_FEAT) for r in res.results
    ]
    out = np.concatenate(shards, axis=0)
    return out, res


def kernel(**inputs):
    out, _ = run(inputs, trace=False)
    return out
